# revision 23
# baseline (speedup 1.0000x reference)
"""GCN block (self-loop + sym-norm + linear + scatter-add + bias + relu) on 8 trn2 cores.

Sharding: nodes partitioned across cores by destination range. Each core
gathers x[src] rows for its incoming edges from a device-side bf16 copy of x
(256B-strided table, 128B payload per descriptor) via SWDGE dma_gather,
aggregates per 128-node destination tile with one-hot matmuls on the tensor
engine, then applies the symmetric normalization and the linear layer.

Host does integer-only graph preprocessing (CSR-style bucketing, degree
counts, padding, int16 index wrapping); all floating-point math runs on
device (including the fp32->bf16 cast of x, done once in a prologue).
"""

import math
import os
import sys

import numpy as np

sys.path.insert(0, "/opt/trn_rl_repo")

import concourse.bacc as bacc
import concourse.bass as bass
import concourse.mybir as mybir
import concourse.tile as tile
from concourse.ap import AP as _AP
from concourse.bass_utils import run_bass_kernel_spmd

F32 = mybir.dt.float32
BF16 = mybir.dt.bfloat16
I16 = mybir.dt.int16

N_CORES = 8
P = 128            # node-tile width / partition count
SUPER = 4          # node tiles per gather super-tile
LO_LIM = 32768     # int16 positive range for gather indices


# ----------------------------------------------------------------------------
# host-side integer preprocessing
# ----------------------------------------------------------------------------

def _wrap_idxs(idx):
    """[n] int16 (n % 128 == 0) -> [128, n//16] wrapped+replicated layout."""
    n = idx.shape[0]
    arr = idx.reshape(n // 16, 16).T  # [16, cols]; arr[p, s] = idx[s*16+p]
    return np.tile(arr, (8, 1))


def preprocess(edge_index, n_nodes):
    """Bucket edges (incl. self-loops) by (core, dest tile); pad to 128-edge
    blocks split into lo/hi src halves. Returns per-core arrays + static meta.
    """
    E = edge_index.shape[1]
    # self-loops are NOT gathered: handled by a per-tile diag(dis) matmul on
    # a contiguous x_own slice (keeps them off the Q7 descriptor-gen path)
    src = np.asarray(edge_index[0])
    dst = np.asarray(edge_index[1])

    deg = np.bincount(dst, minlength=n_nodes).astype(np.int64) + 1  # + loop

    npc = (n_nodes + N_CORES - 1) // N_CORES          # nodes per core
    T = (npc + P - 1) // P                            # tiles per core
    core = dst // npc
    d_local = dst - core * npc
    t_of = d_local // P
    dst_local = d_local % P
    is_lo = src < LO_LIM

    # sort edges by (core, tile, lo/hi) once; stable order inside groups
    order = np.lexsort((~is_lo, t_of, core))
    src_s, core_s, t_s, dl_s, lo_s = (
        src[order], core[order], t_of[order], dst_local[order], is_lo[order])

    # per (core, tile) lo/hi counts -> per-tile block counts (max over cores)
    key = (core_s * T + t_s) * 2 + (~lo_s).astype(np.int64)
    cnt = np.bincount(key, minlength=N_CORES * T * 2).reshape(N_CORES, T, 2)
    nb = np.ceil(cnt / P).astype(np.int64)            # blocks per (c, t, lo/hi)
    nb_lo = nb[:, :, 0].max(axis=0)                   # [T] uniform across cores
    nb_hi = nb[:, :, 1].max(axis=0)                   # [T]

    # super-tile grouping
    supers = [list(range(s, min(s + SUPER, T))) for s in range(0, T, SUPER)]

    # global block layout: per super-tile: [lo blocks by tile][hi blocks by tile]
    # record, per tile: (lo_block_start, nb_lo, hi_block_start, nb_hi) global idx
    blk_of_tile = {}
    calls = []      # per super-tile: (lo_nidx, hi_nidx, blk_start, nb_total)
    B = 0
    for S in supers:
        b0 = B
        lo_starts = {}
        for t in S:
            lo_starts[t] = B
            B += int(nb_lo[t])
        n_lo_blocks = B - b0
        for t in S:
            blk_of_tile[t] = (lo_starts[t], int(nb_lo[t]), B, int(nb_hi[t]))
            B += int(nb_hi[t])
        calls.append((n_lo_blocks * P, (B - b0 - n_lo_blocks) * P, b0, B - b0))
    NB = B  # total blocks per core

    # per-core padded slot arrays
    idx_cols = sum((lo + hi) // 16 for lo, hi, _, _ in calls)
    idx_all = np.zeros((N_CORES, P, idx_cols), np.int16)
    dl_all = np.full((N_CORES, P, NB), -1.0, np.float32)   # cast to bf16 later
    degsrc_all = np.ones((N_CORES, P, NB), np.float32)

    # group boundaries in the sorted edge array
    grp_start = np.zeros(N_CORES * T * 2 + 1, np.int64)
    np.cumsum(np.bincount(key, minlength=N_CORES * T * 2), out=grp_start[1:])

    for c in range(N_CORES):
        col = 0
        for (S, (lo_nidx, hi_nidx, b0, nbS)) in zip(supers, calls):
            for half, nidx in ((0, lo_nidx), (1, hi_nidx)):
                half_start = b0 if half == 0 else b0 + lo_nidx // P
                flat_idx = np.zeros(nidx, np.int64)
                pos = 0
                for t in S:
                    g = (c * T + t) * 2 + half
                    s0, s1 = grp_start[g], grp_start[g + 1]
                    cnt_g = s1 - s0
                    nb_g = int(nb_lo[t] if half == 0 else nb_hi[t])
                    sl = slice(pos, pos + cnt_g)
                    flat_idx[sl] = src_s[s0:s1] - (0 if half == 0 else LO_LIM)
                    # block-slot arrays: slot j -> (partition j%P, block j//P)
                    j = np.arange(pos, pos + cnt_g)
                    bcol = half_start + j // P
                    prow = j % P
                    dl_all[c, prow, bcol] = dl_s[s0:s1]
                    degsrc_all[c, prow, bcol] = deg[src_s[s0:s1]]
                    pos += nb_g * P
                if nidx:
                    idx_all[c, :, col:col + nidx // 16] = _wrap_idxs(
                        flat_idx.astype(np.int16))
                    col += nidx // 16
        assert col == idx_cols

    # per-core degree of own nodes, tile layout [P, T]
    deg_tile = np.ones((N_CORES, P, T), np.float32)
    for c in range(N_CORES):
        n0 = c * npc
        n1 = min(n0 + npc, n_nodes)
        own = deg[n0:n1].astype(np.float32)
        dt = np.ones(T * P, np.float32)
        dt[:own.shape[0]] = own
        deg_tile[c] = dt.reshape(T, P).T

    nbs_max = max(nbS for _, _, _, nbS in calls)
    meta = dict(npc=npc, T=T, NB=NB, idx_cols=idx_cols, supers=supers,
                calls=calls, blk_of_tile=blk_of_tile,
                nb_lo=nb_lo, nb_hi=nb_hi, nbs_max=nbs_max)
    arrays = dict(idx=idx_all, dst_local=dl_all, deg_src=degsrc_all,
                  deg_tile=deg_tile)
    return meta, arrays


# ----------------------------------------------------------------------------
# device program
# ----------------------------------------------------------------------------

def raw_dma_gather(eng, out_ap, in_ap, idxs_ap, num_idxs, elem_size,
                   stride_bytes_256, queue_num):
    """dma_gather with a sub-256B payload (elem_size in elements of the in/out
    dtype); table row stride is stride_bytes_256*256 bytes. Mirrors the bass
    wrapper lowering without its payload-granularity assert."""
    _in_ap = eng.lower_ap_dma(in_ap, for_custom_bir_dma=True)
    _idxs_ap = eng.lower_ap(idxs_ap)
    _out_ap = eng.lower_ap(out_ap)
    return eng.add_instruction(mybir.InstDMAGatherAnt(
        name=eng.bass.get_next_instruction_name(),
        ins=[*_in_ap, _idxs_ap,
             eng.lower_val_access(eng.to_reg(num_idxs))],
        outs=[_out_ap],
        transpose=False, num_idxs=num_idxs, elem_size=elem_size,
        stride_bytes_256=stride_bytes_256, gen_mode=0,
        single_packet=False, queue_num=queue_num,
        sbuf_tokens_per_rank=0, sbuf_free_dim_per_rank=0,
        sbuf_free_dim_pad_per_rank=0, sbuf_byte_offset=0))


def build_nc(n_nodes, d_in, d_out, meta, reps=1, mode="full", max_idx=2048,
             n_queues=4, scratch=16384, gbufs=2, pbufs=2):
    T, NB, idx_cols = meta["T"], meta["NB"], meta["idx_cols"]
    NBS_MAX = meta["nbs_max"]
    out_rows = T * P
    npad = (n_nodes + P - 1) // P * P
    row2 = 2 * d_in          # bf16 table row: 64 data + 64 pad = 256B

    nc = bacc.Bacc("TRN2", target_bir_lowering=False, debug=False,
                   num_swdge_queues=n_queues, dynamic_dma_scratch_size=scratch)

    x_d = nc.dram_tensor("x", [npad, d_in], F32, kind="ExternalInput")
    xown_d = nc.dram_tensor("x_own", [out_rows, d_in], F32, kind="ExternalInput")
    waug_d = nc.dram_tensor("w_aug", [d_in + 1, d_out], F32, kind="ExternalInput")
    # c-major iota: iota_cb[p, c*NBS_MAX + j] = c. Unit-stride last dims keep
    # the one-hot compare/scale in the DVE 2x perf mode.
    iota_d = nc.dram_tensor("iota_cb", [P, P * NBS_MAX], BF16,
                            kind="ExternalInput")
    id01_d = nc.dram_tensor("id01", [P, P], F32, kind="ExternalInput")
    degt_d = nc.dram_tensor("deg_tile", [P, T], F32, kind="ExternalInput")
    degs_d = nc.dram_tensor("deg_src", [P, NB], F32, kind="ExternalInput")
    dl_d = nc.dram_tensor("dst_local", [P, NB], BF16, kind="ExternalInput")
    idx_d = nc.dram_tensor("idx", [P, idx_cols], I16, kind="ExternalInput")
    out_d = nc.dram_tensor("out", [out_rows, d_out], F32, kind="ExternalOutput")

    with tile.TileContext(nc) as tc:
        with (
            tc.tile_pool(name="dram", bufs=1, space="DRAM") as dpool,
            tc.tile_pool(name="const", bufs=1) as cpool,
            tc.tile_pool(name="gather", bufs=gbufs) as gpool,
            tc.tile_pool(name="pmat", bufs=pbufs) as ppool,
            tc.tile_pool(name="small", bufs=5) as spool,
            tc.tile_pool(name="psum", bufs=4, space="PSUM") as psum,
            tc.tile_pool(name="psum2", bufs=2, space="PSUM") as psum2,
        ):
            # ---- bf16 x table: fp32 -> bf16 cast into a 256B-strided DRAM
            # table, once (outside the rep loop) ----
            xb_t = dpool.tile([npad, row2], BF16, tag="xb")
            tcast = npad // P
            CH = 56
            with tc.tile_pool(name="cast", bufs=2) as castp:
                for c0 in range(0, tcast, CH):
                    k = min(CH, tcast - c0)
                    xf = castp.tile([P, CH * d_in], F32, tag="xf")
                    nc.sync.dma_start(
                        xf[:, :k * d_in].rearrange("p (t f) -> p t f", f=d_in),
                        x_d[c0 * P:(c0 + k) * P, :].rearrange(
                            "(t p) f -> p t f", p=P))
                    xc = castp.tile([P, CH * d_in], BF16, tag="xc")
                    nc.scalar.activation(
                        xc[:, :k * d_in], xf[:, :k * d_in],
                        mybir.ActivationFunctionType.Copy)
                    nc.sync.dma_start(
                        xb_t[c0 * P:(c0 + k) * P, :d_in].rearrange(
                            "(t p) f -> p t f", p=P),
                        xc[:, :k * d_in].rearrange("p (t f) -> p t f", f=d_in))

            # ---- constants / one-shot prep ----
            iota_sb = cpool.tile([P, P * NBS_MAX], BF16, tag="iota")
            nc.sync.dma_start(iota_sb[:, :], iota_d[:, :])
            iota3 = iota_sb[:, :].rearrange("p (c j) -> p c j", j=NBS_MAX)
            id01_sb = cpool.tile([P, P], F32, tag="id01")
            nc.sync.dma_start(id01_sb[:, :], id01_d[:, :])
            waug_sb = cpool.tile([d_in + 1, d_out], F32, tag="waug")
            nc.sync.dma_start(waug_sb[:, :], waug_d[:, :])
            dl_sb = cpool.tile([P, NB], BF16, tag="dl")
            nc.sync.dma_start(dl_sb[:, :], dl_d[:, :])
            idx_sb = cpool.tile([P, idx_cols], I16, tag="idx")
            nc.sync.dma_start(idx_sb[:, :], idx_d[:, :])

            def rsqrt(pool, deg_dram, cols, tag):
                """fp32 1/sqrt(deg): ACT sqrt + DVE reciprocal + 1 Newton."""
                d = pool.tile([P, cols], F32, tag=f"{tag}_d")
                nc.sync.dma_start(d[:, :], deg_dram[:, :])
                r = pool.tile([P, cols], F32, tag=f"{tag}_r")
                t1 = pool.tile([P, cols], F32, tag=f"{tag}_t")
                nc.scalar.sqrt(t1[:, :], d[:, :])
                nc.vector.reciprocal(r[:, :], t1[:, :])
                # newton: r <- r * (1.5 - 0.5 * d * r * r)
                nc.vector.tensor_mul(t1[:, :], r[:, :], r[:, :])
                nc.vector.tensor_mul(t1[:, :], t1[:, :], d[:, :])
                nc.vector.tensor_scalar(
                    out=t1[:, :], in0=t1[:, :], scalar1=-0.5, scalar2=1.5,
                    op0=mybir.AluOpType.mult, op1=mybir.AluOpType.add)
                nc.vector.tensor_mul(r[:, :], r[:, :], t1[:, :])
                return r

            dis_dst = rsqrt(cpool, degt_d, T, "degt")           # [P, T] fp32
            dis_src_f = rsqrt(cpool, degs_d, NB, "degs")        # [P, NB] fp32
            dis_src = cpool.tile([P, NB], BF16, tag="dis_src_bf")
            nc.vector.tensor_copy(dis_src[:, :], dis_src_f[:, :])

            ones_row = cpool.tile([1, P], F32, tag="ones")
            nc.vector.memset(ones_row[:, :], 1.0)

            out_sb = cpool.tile([P, T * d_out], F32, tag="out_sb")
            nc.vector.memset(out_sb[:, :], 0.0)

            xo_sb = cpool.tile([P, T * d_in], F32, tag="xo")
            nc.sync.dma_start(
                xo_sb[:, :].rearrange("p (t f) -> p t f", f=d_in),
                xown_d[:, :].rearrange("(t p) f -> p t f", p=P))

            # ---- one-hot scatter matrices (graph constants): build once on
            # DVE, park in DRAM, stream back densely per rep ----
            pw_t = dpool.tile([P, NB * P], BF16, tag="pw_dram")
            pw_dram = pw_t[:, :]
            for S, (lo_nidx, hi_nidx, b0, nbS) in zip(meta["supers"],
                                                      meta["calls"]):
                dl3 = dl_sb[:, b0:b0 + nbS].rearrange(
                    "p (o b) -> p o b", o=1).to_broadcast([P, P, nbS])
                ds3 = dis_src[:, b0:b0 + nbS].rearrange(
                    "p (o b) -> p o b", o=1).to_broadcast([P, P, nbS])
                pwb = ppool.tile([P, P * NBS_MAX], BF16, tag="pw")
                pw3 = pwb[:, :P * nbS].rearrange("p (c b) -> p c b", b=nbS)
                nc.vector.tensor_tensor(
                    out=pw3, in0=dl3, in1=iota3[:, :, :nbS],
                    op=mybir.AluOpType.is_equal)
                nc.vector.tensor_tensor(
                    out=pw3, in0=pw3, in1=ds3, op=mybir.AluOpType.mult)
                nc.sync.dma_start(pw_dram[:, b0 * P:(b0 + nbS) * P],
                                  pwb[:, :P * nbS])

            # per-tile self-loop diagonals (also constant)
            diag_all = cpool.tile([P, T * P], F32, tag="diag_all")
            for t in range(T):
                nc.vector.tensor_scalar_mul(
                    diag_all[:, t * P:(t + 1) * P], id01_sb[:, :],
                    dis_dst[:, t:t + 1])

            # ---- main loop over gather super-tiles ----
            qstate = [0]
            xb_ap = xb_t[:, :]

            def body():
              col = 0
              for S, (lo_nidx, hi_nidx, b0, nbS) in zip(meta["supers"], meta["calls"]):
                xg = gpool.tile([P, nbS * d_in], BF16, tag="xg")
                xg3 = xg[:, :].rearrange("p (b e) -> p b e", e=d_in)
                if mode == "compute":
                    nc.gpsimd.memset(xg[:, :], 0.25)
                # SWDGE descriptor ring; chop each half into <=max_idx calls.
                halves = [(lo_nidx, 0, min(LO_LIM, npad), 0)]
                if hi_nidx:
                    halves.append((hi_nidx, LO_LIM, npad - LO_LIM,
                                   lo_nidx // P))
                for half_nidx, row0, nrows, blk0 in halves:
                    src_ap = _AP(xb_ap.tensor, xb_ap.offset + row0 * row2,
                                 [[row2, nrows], [1, d_in]])
                    nchunk = -(-half_nidx // max_idx)
                    chunk = -(-half_nidx // (nchunk * P)) * P if nchunk else 0
                    done = 0
                    while done < half_nidx:
                        n_i = min(chunk, half_nidx - done)
                        b_lo = blk0 + done // P
                        if mode != "compute":
                            qstate[0] = (qstate[0] + 1) % n_queues
                            raw_dma_gather(
                                nc.gpsimd,
                                out_ap=xg3[:, b_lo:b_lo + n_i // P, :],
                                in_ap=src_ap,
                                idxs_ap=idx_sb[:, col:col + n_i // 16],
                                num_idxs=n_i, elem_size=d_in,
                                stride_bytes_256=1, queue_num=qstate[0])
                        col += n_i // 16
                        done += n_i
                if mode == "gather":
                    continue

                # stream the precomputed one-hot matrices back in (dense DMA
                # rides the idle DMA-engine capacity under the gather)
                pw = ppool.tile([P, P * NBS_MAX], BF16, tag="pw")
                nc.sync.dma_start(pw[:, :P * nbS],
                                  pw_dram[:, b0 * P:(b0 + nbS) * P])
                pw3 = pw[:, :P * nbS].rearrange("p (c b) -> p c b", b=nbS)

                # stage-major emission: all accumulations first, then the
                # per-tile tails, so tail-stage ACT round-trips never stall
                # the PE queue ahead of the next tile's block matmuls
                ps_of, s_of, pst_of, sT_of = {}, {}, {}, {}
                for t in S:
                    lo_b, n_lo, hi_b, n_hi = meta["blk_of_tile"][t]
                    blocks = list(range(lo_b, lo_b + n_lo)) + \
                             list(range(hi_b, hi_b + n_hi))
                    ps = psum.tile([P, d_out], F32, tag="ps")
                    ps_of[t] = ps
                    # self-loop term: psum = diag(dis_dst) @ x_own[tile]
                    nc.tensor.matmul(
                        ps[:, :], lhsT=diag_all[:, t * P:(t + 1) * P],
                        rhs=xo_sb[:, t * d_in:(t + 1) * d_in],
                        start=True, stop=(len(blocks) == 0))
                    for i, b in enumerate(blocks):
                        rb = b - b0
                        nc.tensor.matmul(
                            ps[:, :],
                            lhsT=pw3[:, :, rb],
                            rhs=xg[:, rb * d_in:(rb + 1) * d_in],
                            start=False, stop=(i == len(blocks) - 1))
                for t in S:
                    # s -> sbuf scaled by dis[dst] (per-partition ACT scale)
                    s_sb = spool.tile([P, d_out], F32, tag="s_sb")
                    s_of[t] = s_sb
                    nc.scalar.activation(
                        s_sb[:, :], ps_of[t][:, :],
                        mybir.ActivationFunctionType.Copy,
                        scale=dis_dst[:, t:t + 1])
                for t in S:
                    # transpose on PE with a true identity
                    pst = psum2.tile([d_out, P], F32, tag="pst")
                    pst_of[t] = pst
                    nc.tensor.transpose(pst[:, :], s_of[t][:, :], id01_sb[:, :])
                for t in S:
                    sT = spool.tile([d_in + 1, P], F32, tag="sT")
                    sT_of[t] = sT
                    nc.scalar.activation(
                        sT[:d_out, :], pst_of[t][:, :],
                        mybir.ActivationFunctionType.Copy)
                    nc.vector.tensor_copy(sT[d_in:d_in + 1, :], ones_row[:, :])
                for t in S:
                    po = psum2.tile([P, d_out], F32, tag="po")
                    nc.tensor.matmul(po[:, :], lhsT=sT_of[t][:, :],
                                     rhs=waug_sb[:, :], start=True, stop=True)
                    nc.scalar.activation(
                        out_sb[:, t * d_out:(t + 1) * d_out], po[:, :],
                        mybir.ActivationFunctionType.Relu)

            if reps == 1:
                body()
            else:
                with tc.For_i(0, reps, 1):
                    body()

            nc.sync.dma_start(
                out_d[:, :].rearrange("(t p) f -> p t f", p=P),
                out_sb[:, :].rearrange("p (t f) -> p t f", f=d_out))

    nc.compile()
    return nc


# ----------------------------------------------------------------------------
# public entry point
# ----------------------------------------------------------------------------

_CACHE = {}


def _get_compiled(n_nodes, d_in, d_out, edge_index):
    key = (n_nodes, d_in, d_out,
           hash(edge_index.tobytes()) if edge_index.size < (1 << 24)
           else hash(edge_index[:, ::97].tobytes()))
    hit = _CACHE.get(key)
    if hit is None:
        meta, arrays = preprocess(np.asarray(edge_index, dtype=np.int64), n_nodes)
        nc = build_nc(n_nodes, d_in, d_out, meta)
        hit = (nc, meta, arrays)
        _CACHE[key] = hit
    return hit


def _make_in_maps(x, W, b, meta, arrays, d_in, d_out):
    w_aug = np.concatenate([np.asarray(W, np.float32),
                            np.asarray(b, np.float32)[None, :]], axis=0)
    import ml_dtypes
    nbs_max = meta["nbs_max"]
    n = x.shape[0]
    npad = (n + P - 1) // P * P
    xpad = np.zeros((npad, x.shape[1]), np.float32)
    xpad[:n] = np.asarray(x, np.float32)
    iota_cb = np.repeat(np.arange(P, dtype=np.float32), nbs_max)  # [P*nbs_max]
    iota_cb = np.tile(iota_cb, (P, 1)).astype(ml_dtypes.bfloat16)
    id01 = np.eye(P, dtype=np.float32)
    in_maps = []
    for c in range(N_CORES):
        xo = np.zeros((meta["T"] * P, x.shape[1]), np.float32)
        n0 = c * meta["npc"]
        n1 = min(n0 + meta["npc"], n)
        xo[:n1 - n0] = np.asarray(x[n0:n1], np.float32)
        in_maps.append({
            "x": xpad,
            "x_own": xo,
            "w_aug": w_aug,
            "iota_cb": iota_cb,
            "id01": id01,
            "deg_tile": arrays["deg_tile"][c],
            "deg_src": arrays["deg_src"][c],
            "dst_local": arrays["dst_local"][c].astype(ml_dtypes.bfloat16),
            "idx": arrays["idx"][c],
        })
    return in_maps


def run(x, edge_index, W, b, trace=False):
    n_nodes, d_in = x.shape
    d_out = W.shape[1]
    nc, meta, arrays = _get_compiled(n_nodes, d_in, d_out,
                                     np.asarray(edge_index, dtype=np.int64))
    in_maps = _make_in_maps(x, W, b, meta, arrays, d_in, d_out)
    res = run_bass_kernel_spmd(nc, in_maps, core_ids=list(range(N_CORES)),
                               trace=trace)
    npc = meta["npc"]
    parts = [res.results[c]["out"][:min(npc, n_nodes - c * npc)]
             for c in range(N_CORES)]
    out = np.concatenate(parts, axis=0).astype(np.float32)
    return out, res


def kernel(x, edge_index, W, b):
    out, _ = run(np.asarray(x), np.asarray(edge_index), np.asarray(W),
                 np.asarray(b))
    return out


# revision 30
# speedup vs baseline: 1.1331x; 1.1331x over previous
"""GCN block (self-loop + sym-norm + linear + scatter-add + bias + relu) on 8 trn2 cores.

Sharding: nodes partitioned across cores by destination range. Each core
gathers x[src] rows for its incoming edges from a device-side bf16 copy of x
(256B-strided table, 128B payload per descriptor) via SWDGE dma_gather,
aggregates per 128-node destination tile with one-hot matmuls on the tensor
engine, then applies the symmetric normalization and the linear layer.

Host does integer-only graph preprocessing (CSR-style bucketing, degree
counts, padding, int16 index wrapping); all floating-point math runs on
device (including the fp32->bf16 cast of x, done once in a prologue).
"""

import math
import os
import sys

import numpy as np

sys.path.insert(0, "/opt/trn_rl_repo")

import concourse.bacc as bacc
import concourse.bass as bass
import concourse.mybir as mybir
import concourse.tile as tile
from concourse.ap import AP as _AP
from concourse.bass_utils import run_bass_kernel_spmd

F32 = mybir.dt.float32
BF16 = mybir.dt.bfloat16
I16 = mybir.dt.int16
FP8 = mybir.dt.float8e4

N_CORES = 8
P = 128            # node-tile width / partition count
SUPER = 4          # node tiles per gather super-tile
LO_LIM = 32768     # int16 positive range for gather indices


# ----------------------------------------------------------------------------
# host-side integer preprocessing
# ----------------------------------------------------------------------------

def _wrap_idxs(idx):
    """[n] int16 (n % 128 == 0) -> [128, n//16] wrapped+replicated layout."""
    n = idx.shape[0]
    arr = idx.reshape(n // 16, 16).T  # [16, cols]; arr[p, s] = idx[s*16+p]
    return np.tile(arr, (8, 1))


def preprocess(edge_index, n_nodes):
    """Bucket edges (incl. self-loops) by (core, dest tile); pad to 128-edge
    blocks split into lo/hi src halves. Returns per-core arrays + static meta.
    """
    E = edge_index.shape[1]
    # self-loops are NOT gathered: handled by a per-tile diag(dis) matmul on
    # a contiguous x_own slice (keeps them off the Q7 descriptor-gen path)
    src = np.asarray(edge_index[0])
    dst = np.asarray(edge_index[1])

    deg = np.bincount(dst, minlength=n_nodes).astype(np.int64) + 1  # + loop

    npc = (n_nodes + N_CORES - 1) // N_CORES          # nodes per core
    T = (npc + P - 1) // P                            # tiles per core
    core = dst // npc
    d_local = dst - core * npc
    t_of = d_local // P
    dst_local = d_local % P
    is_lo = src < LO_LIM

    # sort edges by (core, tile, lo/hi) once; stable order inside groups
    order = np.lexsort((~is_lo, t_of, core))
    src_s, core_s, t_s, dl_s, lo_s = (
        src[order], core[order], t_of[order], dst_local[order], is_lo[order])

    # per (core, tile) lo/hi counts -> per-tile block counts (max over cores)
    key = (core_s * T + t_s) * 2 + (~lo_s).astype(np.int64)
    cnt = np.bincount(key, minlength=N_CORES * T * 2).reshape(N_CORES, T, 2)
    nb = np.ceil(cnt / P).astype(np.int64)            # blocks per (c, t, lo/hi)
    nb_lo = nb[:, :, 0].max(axis=0)                   # [T] uniform across cores
    nb_hi = nb[:, :, 1].max(axis=0)                   # [T]

    # super-tile grouping
    supers = [list(range(s, min(s + SUPER, T))) for s in range(0, T, SUPER)]

    # global block layout: per super-tile: [lo blocks by tile][hi blocks by tile]
    # record, per tile: (lo_block_start, nb_lo, hi_block_start, nb_hi) global idx
    blk_of_tile = {}
    calls = []      # per super-tile: (lo_nidx, hi_nidx, blk_start, nb_total)
    B = 0
    for S in supers:
        b0 = B
        lo_starts = {}
        for t in S:
            lo_starts[t] = B
            B += int(nb_lo[t])
        n_lo_blocks = B - b0
        for t in S:
            blk_of_tile[t] = (lo_starts[t], int(nb_lo[t]), B, int(nb_hi[t]))
            B += int(nb_hi[t])
        calls.append((n_lo_blocks * P, (B - b0 - n_lo_blocks) * P, b0, B - b0))
    NB = B  # total blocks per core

    # per-core padded slot arrays
    idx_cols = sum((lo + hi) // 16 for lo, hi, _, _ in calls)
    idx_all = np.zeros((N_CORES, P, idx_cols), np.int16)
    dl_all = np.full((N_CORES, P, NB), -1.0, np.float32)   # cast to bf16 later
    degsrc_all = np.ones((N_CORES, P, NB), np.float32)

    # group boundaries in the sorted edge array
    grp_start = np.zeros(N_CORES * T * 2 + 1, np.int64)
    np.cumsum(np.bincount(key, minlength=N_CORES * T * 2), out=grp_start[1:])

    for c in range(N_CORES):
        col = 0
        for (S, (lo_nidx, hi_nidx, b0, nbS)) in zip(supers, calls):
            for half, nidx in ((0, lo_nidx), (1, hi_nidx)):
                half_start = b0 if half == 0 else b0 + lo_nidx // P
                flat_idx = np.zeros(nidx, np.int64)
                pos = 0
                for t in S:
                    g = (c * T + t) * 2 + half
                    s0, s1 = grp_start[g], grp_start[g + 1]
                    cnt_g = s1 - s0
                    nb_g = int(nb_lo[t] if half == 0 else nb_hi[t])
                    sl = slice(pos, pos + cnt_g)
                    flat_idx[sl] = src_s[s0:s1] - (0 if half == 0 else LO_LIM)
                    # block-slot arrays: slot j -> (partition j%P, block j//P)
                    j = np.arange(pos, pos + cnt_g)
                    bcol = half_start + j // P
                    prow = j % P
                    dl_all[c, prow, bcol] = dl_s[s0:s1]
                    degsrc_all[c, prow, bcol] = deg[src_s[s0:s1]]
                    pos += nb_g * P
                if nidx:
                    idx_all[c, :, col:col + nidx // 16] = _wrap_idxs(
                        flat_idx.astype(np.int16))
                    col += nidx // 16
        assert col == idx_cols

    # per-core degree of own nodes, tile layout [P, T]
    deg_tile = np.ones((N_CORES, P, T), np.float32)
    for c in range(N_CORES):
        n0 = c * npc
        n1 = min(n0 + npc, n_nodes)
        own = deg[n0:n1].astype(np.float32)
        dt = np.ones(T * P, np.float32)
        dt[:own.shape[0]] = own
        deg_tile[c] = dt.reshape(T, P).T

    # global per-node degrees, padded + tiled [P, npad//P] (for the
    # dis-scaled bf16 gather table built on device)
    npad = (n_nodes + P - 1) // P * P
    degpad = np.ones(npad, np.float32)
    degpad[:n_nodes] = deg.astype(np.float32)
    deg_all = degpad.reshape(npad // P, P).T

    nbs_max = max(nbS for _, _, _, nbS in calls)
    meta = dict(npc=npc, T=T, NB=NB, idx_cols=idx_cols, supers=supers,
                calls=calls, blk_of_tile=blk_of_tile,
                nb_lo=nb_lo, nb_hi=nb_hi, nbs_max=nbs_max)
    arrays = dict(idx=idx_all, dst_local=dl_all, deg_src=degsrc_all,
                  deg_tile=deg_tile, deg_all=deg_all)
    return meta, arrays


# ----------------------------------------------------------------------------
# device program
# ----------------------------------------------------------------------------

def raw_dma_gather(eng, out_ap, in_ap, idxs_ap, num_idxs, elem_size,
                   stride_bytes_256, queue_num):
    """dma_gather with a sub-256B payload (elem_size in elements of the in/out
    dtype); table row stride is stride_bytes_256*256 bytes. Mirrors the bass
    wrapper lowering without its payload-granularity assert."""
    _in_ap = eng.lower_ap_dma(in_ap, for_custom_bir_dma=True)
    _idxs_ap = eng.lower_ap(idxs_ap)
    _out_ap = eng.lower_ap(out_ap)
    return eng.add_instruction(mybir.InstDMAGatherAnt(
        name=eng.bass.get_next_instruction_name(),
        ins=[*_in_ap, _idxs_ap,
             eng.lower_val_access(eng.to_reg(num_idxs))],
        outs=[_out_ap],
        transpose=False, num_idxs=num_idxs, elem_size=elem_size,
        stride_bytes_256=stride_bytes_256, gen_mode=0,
        single_packet=False, queue_num=queue_num,
        sbuf_tokens_per_rank=0, sbuf_free_dim_per_rank=0,
        sbuf_free_dim_pad_per_rank=0, sbuf_byte_offset=0))


def build_nc(n_nodes, d_in, d_out, meta, reps=1, mode="full", max_idx=2048,
             n_queues=4, scratch=16384, gbufs=2, pbufs=2, queue_map=None):
    T, NB, idx_cols = meta["T"], meta["NB"], meta["idx_cols"]
    NBS_MAX = meta["nbs_max"]
    out_rows = T * P
    npad = (n_nodes + P - 1) // P * P
    row2 = 2 * d_in          # bf16 table row: 64 data + 64 pad = 256B

    nc = bacc.Bacc("TRN2", target_bir_lowering=False, debug=False,
                   num_swdge_queues=n_queues, dynamic_dma_scratch_size=scratch)

    x_d = nc.dram_tensor("x", [npad, d_in], F32, kind="ExternalInput")
    xown_d = nc.dram_tensor("x_own", [out_rows, d_in], F32, kind="ExternalInput")
    waug_d = nc.dram_tensor("w_aug", [d_in + 1, d_out], F32, kind="ExternalInput")
    # c-major iota: iota_cb[p, c*8 + j] = c. Unit-stride last dims keep
    # the one-hot compare in the DVE 2x perf mode.
    iota_d = nc.dram_tensor("iota_cb", [P, P * 8], BF16,
                            kind="ExternalInput")
    id01_d = nc.dram_tensor("id01", [P, P], F32, kind="ExternalInput")
    degt_d = nc.dram_tensor("deg_tile", [P, T], F32, kind="ExternalInput")
    dega_d = nc.dram_tensor("deg_all", [P, npad // P], F32,
                            kind="ExternalInput")
    dl_d = nc.dram_tensor("dst_local", [P, NB], BF16, kind="ExternalInput")
    idx_d = nc.dram_tensor("idx", [P, idx_cols], I16, kind="ExternalInput")
    out_d = nc.dram_tensor("out", [out_rows, d_out], F32, kind="ExternalOutput")

    with tile.TileContext(nc) as tc:
        with (
            tc.tile_pool(name="dram", bufs=1, space="DRAM") as dpool,
            tc.tile_pool(name="const", bufs=1) as cpool,
            tc.tile_pool(name="gather", bufs=gbufs) as gpool,
            tc.tile_pool(name="small", bufs=5) as spool,
            tc.tile_pool(name="psum", bufs=4, space="PSUM") as psum,
            tc.tile_pool(name="psum2", bufs=2, space="PSUM") as psum2,
        ):

            # ---- constants / one-shot prep ----
            iota_sb = cpool.tile([P, P * 8], BF16, tag="iota")
            nc.sync.dma_start(iota_sb[:, :], iota_d[:, :])
            iota3 = iota_sb[:, :].rearrange("p (c j) -> p c j", j=8)
            id01_sb = cpool.tile([P, P], F32, tag="id01")
            nc.sync.dma_start(id01_sb[:, :], id01_d[:, :])
            waug_sb = cpool.tile([d_in + 1, d_out], F32, tag="waug")
            nc.sync.dma_start(waug_sb[:, :], waug_d[:, :])
            dl_sb = cpool.tile([P, NB], BF16, tag="dl")
            nc.sync.dma_start(dl_sb[:, :], dl_d[:, :])
            idx_sb = cpool.tile([P, idx_cols], I16, tag="idx")
            nc.sync.dma_start(idx_sb[:, :], idx_d[:, :])

            def rsqrt(pool, deg_dram, cols, tag):
                """fp32 1/sqrt(deg): ACT sqrt + DVE reciprocal + 1 Newton."""
                d = pool.tile([P, cols], F32, tag=f"{tag}_d")
                nc.sync.dma_start(d[:, :], deg_dram[:, :])
                r = pool.tile([P, cols], F32, tag=f"{tag}_r")
                t1 = pool.tile([P, cols], F32, tag=f"{tag}_t")
                nc.scalar.sqrt(t1[:, :], d[:, :])
                nc.vector.reciprocal(r[:, :], t1[:, :])
                # newton: r <- r * (1.5 - 0.5 * d * r * r)
                nc.vector.tensor_mul(t1[:, :], r[:, :], r[:, :])
                nc.vector.tensor_mul(t1[:, :], t1[:, :], d[:, :])
                nc.vector.tensor_scalar(
                    out=t1[:, :], in0=t1[:, :], scalar1=-0.5, scalar2=1.5,
                    op0=mybir.AluOpType.mult, op1=mybir.AluOpType.add)
                nc.vector.tensor_mul(r[:, :], r[:, :], t1[:, :])
                return r

            dis_dst = rsqrt(cpool, degt_d, T, "degt")           # [P, T] fp32
            dis_all = rsqrt(cpool, dega_d, npad // P, "dega")   # [P, npad/P]

            # ---- y table: bf16 dis[u]*x[u] rows in a 256B-strided DRAM
            # table, built once (outside the rep loop). Folding dis[src]
            # here makes the scatter one-hot pure 0/1 (exact in fp8).
            xb_t = dpool.tile([npad, row2], BF16, tag="xb")
            tcast = npad // P
            CH = 16
            with tc.tile_pool(name="cast", bufs=2) as castp:
                for c0 in range(0, tcast, CH):
                    k = min(CH, tcast - c0)
                    xf = castp.tile([P, CH * d_in], F32, tag="xf")
                    nc.sync.dma_start(
                        xf[:, :k * d_in].rearrange("p (t f) -> p t f", f=d_in),
                        x_d[c0 * P:(c0 + k) * P, :].rearrange(
                            "(t p) f -> p t f", p=P))
                    xc = castp.tile([P, CH * d_in], BF16, tag="xc")
                    for t in range(k):
                        nc.scalar.activation(
                            xc[:, t * d_in:(t + 1) * d_in],
                            xf[:, t * d_in:(t + 1) * d_in],
                            mybir.ActivationFunctionType.Copy,
                            scale=dis_all[:, c0 + t:c0 + t + 1])
                    nc.sync.dma_start(
                        xb_t[c0 * P:(c0 + k) * P, :d_in].rearrange(
                            "(t p) f -> p t f", p=P),
                        xc[:, :k * d_in].rearrange("p (t f) -> p t f", f=d_in))

            ones_row = cpool.tile([1, P], F32, tag="ones")
            nc.vector.memset(ones_row[:, :], 1.0)

            out_sb = cpool.tile([P, T * d_out], F32, tag="out_sb")
            nc.vector.memset(out_sb[:, :], 0.0)

            xo_sb = cpool.tile([P, T * d_in], F32, tag="xo")
            nc.sync.dma_start(
                xo_sb[:, :].rearrange("p (t f) -> p t f", f=d_in),
                xown_d[:, :].rearrange("(t p) f -> p t f", p=P))

            # ---- 0/1 one-hot scatter matrices (graph constants): built
            # once on DVE straight into a resident fp8 SBUF tile — no
            # per-rep traffic at all ----
            pw_res = cpool.tile([P, NB * P], FP8, tag="pw_res")
            for S, (lo_nidx, hi_nidx, b0, nbS) in zip(meta["supers"],
                                                      meta["calls"]):
                pwr3 = pw_res[:, b0 * P:(b0 + nbS) * P].rearrange(
                    "p (c b) -> p c b", b=nbS)
                for j0 in range(0, nbS, 8):
                    jw = min(8, nbS - j0)
                    dl3 = dl_sb[:, b0 + j0:b0 + j0 + jw].rearrange(
                        "p (o b) -> p o b", o=1).to_broadcast([P, P, jw])
                    nc.vector.tensor_tensor(
                        out=pwr3[:, :, j0:j0 + jw], in0=dl3,
                        in1=iota3[:, :, :jw],
                        op=mybir.AluOpType.is_equal)

            # ---- main loop over gather super-tiles ----
            gather_insts = []
            xb_ap = xb_t[:, :]

            def body():
              col = 0
              for S, (lo_nidx, hi_nidx, b0, nbS) in zip(meta["supers"], meta["calls"]):
                xg = gpool.tile([P, nbS * d_in], BF16, tag="xg")
                xg3 = xg[:, :].rearrange("p (b e) -> p b e", e=d_in)
                if mode == "compute":
                    nc.gpsimd.memset(xg[:, :], 0.25)
                # SWDGE descriptor ring; chop each half into <=max_idx calls.
                halves = [(lo_nidx, 0, min(LO_LIM, npad), 0)]
                if hi_nidx:
                    halves.append((hi_nidx, LO_LIM, npad - LO_LIM,
                                   lo_nidx // P))
                for half_nidx, row0, nrows, blk0 in halves:
                    src_ap = _AP(xb_ap.tensor, xb_ap.offset + row0 * row2,
                                 [[row2, nrows], [1, d_in]])
                    nchunk = -(-half_nidx // max_idx)
                    chunk = -(-half_nidx // (nchunk * P)) * P if nchunk else 0
                    done = 0
                    while done < half_nidx:
                        n_i = min(chunk, half_nidx - done)
                        b_lo = blk0 + done // P
                        if mode != "compute":
                            k = len(gather_insts)
                            q = (queue_map[k] if queue_map is not None
                                 else (k + 1) % n_queues)
                            gather_insts.append(raw_dma_gather(
                                nc.gpsimd,
                                out_ap=xg3[:, b_lo:b_lo + n_i // P, :],
                                in_ap=src_ap,
                                idxs_ap=idx_sb[:, col:col + n_i // 16],
                                num_idxs=n_i, elem_size=d_in,
                                stride_bytes_256=1, queue_num=q))
                        col += n_i // 16
                        done += n_i
                if mode == "gather":
                    continue

                pw3 = pw_res[:, b0 * P:(b0 + nbS) * P].rearrange(
                    "p (c b) -> p c b", b=nbS)

                # stage-major emission: all accumulations first, then the
                # per-tile tails, so tail-stage ACT round-trips never stall
                # the PE queue ahead of the next tile's block matmuls
                ps_of, s_of, pst_of, sT_of = {}, {}, {}, {}
                for t in S:
                    lo_b, n_lo, hi_b, n_hi = meta["blk_of_tile"][t]
                    blocks = list(range(lo_b, lo_b + n_lo)) + \
                             list(range(hi_b, hi_b + n_hi))
                    ps = psum.tile([P, d_out], F32, tag="ps")
                    ps_of[t] = ps
                    # self-loop term: psum = diag(dis_dst) @ x_own[tile]
                    diag = spool.tile([P, P], F32, tag="diag")
                    nc.vector.tensor_scalar_mul(
                        diag[:, :], id01_sb[:, :], dis_dst[:, t:t + 1])
                    nc.tensor.matmul(
                        ps[:, :], lhsT=diag[:, :],
                        rhs=xo_sb[:, t * d_in:(t + 1) * d_in],
                        start=True, stop=(len(blocks) == 0 or
                                          mode in ("full_nomm", "full_nopw")))
                    if mode in ("full_nomm", "full_nopw"):
                        blocks = []
                    elif mode == "full_half":
                        blocks = blocks[::2]
                    for i, b in enumerate(blocks):
                        rb = b - b0
                        nc.tensor.matmul(
                            ps[:, :],
                            lhsT=pw3[:, :, rb],
                            rhs=xg[:, rb * d_in:(rb + 1) * d_in],
                            start=False, stop=(i == len(blocks) - 1))
                for t in S:
                    # s -> sbuf scaled by dis[dst] (per-partition ACT scale)
                    s_sb = spool.tile([P, d_out], F32, tag="s_sb")
                    s_of[t] = s_sb
                    nc.scalar.activation(
                        s_sb[:, :], ps_of[t][:, :],
                        mybir.ActivationFunctionType.Copy,
                        scale=dis_dst[:, t:t + 1])
                for t in S:
                    # transpose on PE with a true identity
                    pst = psum2.tile([d_out, P], F32, tag="pst")
                    pst_of[t] = pst
                    nc.tensor.transpose(pst[:, :], s_of[t][:, :], id01_sb[:, :])
                for t in S:
                    sT = spool.tile([d_in + 1, P], F32, tag="sT")
                    sT_of[t] = sT
                    nc.scalar.activation(
                        sT[:d_out, :], pst_of[t][:, :],
                        mybir.ActivationFunctionType.Copy)
                    nc.vector.tensor_copy(sT[d_in:d_in + 1, :], ones_row[:, :])
                for t in S:
                    po = psum2.tile([P, d_out], F32, tag="po")
                    nc.tensor.matmul(po[:, :], lhsT=sT_of[t][:, :],
                                     rhs=waug_sb[:, :], start=True, stop=True)
                    nc.scalar.activation(
                        out_sb[:, t * d_out:(t + 1) * d_out], po[:, :],
                        mybir.ActivationFunctionType.Relu)

            if reps == 1:
                body()
            else:
                with tc.For_i(0, reps, 1):
                    body()

            nc.sync.dma_start(
                out_d[:, :].rearrange("(t p) f -> p t f", p=P),
                out_sb[:, :].rearrange("p (t f) -> p t f", f=d_out))

    nc.compile()
    nc._gather_insts = gather_insts
    return nc


_DMASW0_PROC = 11  # PROC_NAMES.index("DMASW0")


def build_nc_aligned(*args, **kw):
    """Two-pass build: SWDGE completion sems are bound to the DMASW lane the
    tile scheduler assigns (round-robin in schedule order), and the ucode
    requires each sem to be updated from a single SWDGE queue. Read the lane
    of every gather from a first build, then rebuild with queue = lane % 4
    so the binding is consistent no matter how the scheduler reordered."""
    n_queues = kw.get("n_queues", 4)
    nc1 = build_nc(*args, **kw)
    lanes = [i.ins.bass_scheduled_proc for i in nc1._gather_insts]
    if not lanes:
        return nc1
    qmap = {k: (l - _DMASW0_PROC) % n_queues for k, l in enumerate(lanes)}
    nc2 = build_nc(*args, queue_map=qmap, **kw)
    lanes2 = [i.ins.bass_scheduled_proc for i in nc2._gather_insts]
    if lanes2 != lanes:
        # schedule shifted; one more fixpoint attempt
        qmap = {k: (l - _DMASW0_PROC) % n_queues
                for k, l in enumerate(lanes2)}
        nc2 = build_nc(*args, queue_map=qmap, **kw)
    return nc2


# ----------------------------------------------------------------------------
# public entry point
# ----------------------------------------------------------------------------

_CACHE = {}


def _get_compiled(n_nodes, d_in, d_out, edge_index):
    key = (n_nodes, d_in, d_out,
           hash(edge_index.tobytes()) if edge_index.size < (1 << 24)
           else hash(edge_index[:, ::97].tobytes()))
    hit = _CACHE.get(key)
    if hit is None:
        meta, arrays = preprocess(np.asarray(edge_index, dtype=np.int64), n_nodes)
        nc = build_nc_aligned(n_nodes, d_in, d_out, meta)
        hit = (nc, meta, arrays)
        _CACHE[key] = hit
    return hit


def _make_in_maps(x, W, b, meta, arrays, d_in, d_out):
    w_aug = np.concatenate([np.asarray(W, np.float32),
                            np.asarray(b, np.float32)[None, :]], axis=0)
    import ml_dtypes
    n = x.shape[0]
    npad = (n + P - 1) // P * P
    xpad = np.zeros((npad, x.shape[1]), np.float32)
    xpad[:n] = np.asarray(x, np.float32)
    iota_cb = np.repeat(np.arange(P, dtype=np.float32), 8)  # [P*8]
    iota_cb = np.tile(iota_cb, (P, 1)).astype(ml_dtypes.bfloat16)
    id01 = np.eye(P, dtype=np.float32)
    in_maps = []
    for c in range(N_CORES):
        xo = np.zeros((meta["T"] * P, x.shape[1]), np.float32)
        n0 = c * meta["npc"]
        n1 = min(n0 + meta["npc"], n)
        xo[:n1 - n0] = np.asarray(x[n0:n1], np.float32)
        in_maps.append({
            "x": xpad,
            "x_own": xo,
            "w_aug": w_aug,
            "iota_cb": iota_cb,
            "id01": id01,
            "deg_tile": arrays["deg_tile"][c],
            "deg_all": arrays["deg_all"],
            "dst_local": arrays["dst_local"][c].astype(ml_dtypes.bfloat16),
            "idx": arrays["idx"][c],
        })
    return in_maps


def run(x, edge_index, W, b, trace=False):
    n_nodes, d_in = x.shape
    d_out = W.shape[1]
    nc, meta, arrays = _get_compiled(n_nodes, d_in, d_out,
                                     np.asarray(edge_index, dtype=np.int64))
    in_maps = _make_in_maps(x, W, b, meta, arrays, d_in, d_out)
    res = run_bass_kernel_spmd(nc, in_maps, core_ids=list(range(N_CORES)),
                               trace=trace)
    npc = meta["npc"]
    parts = [res.results[c]["out"][:min(npc, n_nodes - c * npc)]
             for c in range(N_CORES)]
    out = np.concatenate(parts, axis=0).astype(np.float32)
    return out, res


def kernel(x, edge_index, W, b):
    out, _ = run(np.asarray(x), np.asarray(edge_index), np.asarray(W),
                 np.asarray(b))
    return out
